# revision 1
# baseline (speedup 1.0000x reference)
"""Bidirectional 2-layer LSTM encoder (B=64, T=2048, F=88, H=128) on 8
Trainium2 NeuronCores via Bass/Tile.

Sharding: data-parallel over batch x direction. Cores 0-3 run the forward
direction on 16 batch rows each; cores 4-7 run the backward direction
(time-reversed input) on the same row slices. Each core executes BOTH
layers of its direction as two staggered LSTM cells (layer 1 lags layer 0
by one G-step window), so there is no cross-core communication.

Per-core kernel (per step): 8 recurrent [128x128]@[128,b] bf16 matmuls
accumulate into a PSUM window tile [128, G, 8, b] pre-filled by batched
input GEMMs (lift fused into layer-0 weights on host; biases via a ones
row / rank-1 matmul); gates computed by 2 ACT ops (sigmoid over i,f,o of
both cells; tanh over g) + tanh(c); c/h updates on DVE/GPSIMD.
h is bf16; c accumulates in fp32.
"""

from contextlib import ExitStack

import numpy as np
import ml_dtypes

import concourse.bacc as bacc
import concourse.tile as tile
from concourse import mybir
from concourse.bass_utils import run_bass_kernel_spmd

BF16 = mybir.dt.bfloat16
F32 = mybir.dt.float32
AF = mybir.ActivationFunctionType

H = 128
FIN = 88
KX = FIN + 1  # x rows + ones row
B_FULL = 64
T_FULL = 2048
B_CORE = 16
N_CORES = 8

_NC_CACHE = {}


def build(T=T_FULL, b=B_CORE, G=4, use_gp=True, num_devices=N_CORES):
    """Build the per-core Bass program (fully unrolled)."""
    assert T % G == 0 and G % 2 == 0
    W = T // G  # cell0 windows; wall windows 0..W inclusive
    nc = bacc.Bacc("TRN2", target_bir_lowering=False, debug=False,
                   num_devices=num_devices)

    xT = nc.dram_tensor("xT", [KX, T, b], BF16, kind="ExternalInput").ap()
    m0 = nc.dram_tensor("m0", [KX, 4 * H], BF16, kind="ExternalInput").ap()
    u0 = nc.dram_tensor("u0", [H, 4 * H], BF16, kind="ExternalInput").ap()
    w1 = nc.dram_tensor("w1", [H, 4 * H], BF16, kind="ExternalInput").ap()
    u1 = nc.dram_tensor("u1", [H, 4 * H], BF16, kind="ExternalInput").ap()
    b1v = nc.dram_tensor("b1v", [1, 4 * H], BF16, kind="ExternalInput").ap()
    oh = nc.dram_tensor("oh", [H, T, b], BF16, kind="ExternalOutput").ap()
    fcs = nc.dram_tensor("fcs", [2, H, b], F32, kind="ExternalOutput").ap()

    # gate chunk (keras order i,f,g,o) -> (cell0 slot, cell1 slot)
    CHUNK_SLOT = {0: (0, 1), 1: (2, 3), 3: (4, 5), 2: (6, 7)}

    with tile.TileContext(nc) as tc, ExitStack() as ctx:
        wts = ctx.enter_context(tc.tile_pool(name="wts", bufs=1))
        state = ctx.enter_context(tc.tile_pool(name="state", bufs=1))
        big = ctx.enter_context(tc.tile_pool(name="big", bufs=1))
        psum = ctx.enter_context(tc.tile_pool(name="psum", bufs=3, space="PSUM"))
        act = ctx.enter_context(tc.tile_pool(name="act", bufs=4))
        tmp = ctx.enter_context(tc.tile_pool(name="tmp", bufs=4))

        m0s = wts.tile([KX, 4 * H], BF16)
        u0s = wts.tile([H, 4 * H], BF16)
        w1s = wts.tile([H, 4 * H], BF16)
        u1s = wts.tile([H, 4 * H], BF16)
        b1s = wts.tile([1, 4 * H], BF16)
        ones = wts.tile([1, G * b], BF16)
        nc.sync.dma_start(out=m0s[:], in_=m0[:])
        nc.sync.dma_start(out=u0s[:], in_=u0[:])
        nc.sync.dma_start(out=w1s[:], in_=w1[:])
        nc.sync.dma_start(out=u1s[:], in_=u1[:])
        nc.sync.dma_start(out=b1s[:], in_=b1v[:])
        nc.vector.memset(ones[:], 1.0)

        xs = big.tile([KX, T, b], BF16)
        nc.sync.dma_start(out=xs[:], in_=xT[:])
        ohb = big.tile([H, T, b], BF16)

        ring = state.tile([H, 2, G, 2, b], BF16)  # [*, parity, step, cell, b]
        nc.vector.memset(ring[:], 0.0)
        cA = state.tile([H, 2, b], F32)
        cB = state.tile([H, 2, b], F32)
        nc.vector.memset(cA[:], 0.0)

        def fill_window(w):
            pz = psum.tile([H, G, 8, b], F32)
            t0 = w * G
            rpar_prev = (w - 1) % 2
            # start=True clears has_written for the WHOLE bank -> only the
            # first matmul into this tile may use it.
            first = [True]

            def st():
                v = first[0]
                first[0] = False
                return v

            for j in range(4):
                s0, s1 = CHUNK_SLOT[j]
                wcol = slice(j * H, (j + 1) * H)
                if w < W:
                    # x~0 for cell0 steps t0..t0+G (K=89 incl. bias row)
                    nc.tensor.matmul(
                        out=pz[:, :, s0, :], lhsT=m0s[:, wcol],
                        rhs=xs[:, t0:t0 + G, :],
                        start=st(), stop=False, skip_group_check=True)
                # x~1 for cell1 steps (w-1)G..wG from h0 history ring
                # (w=0: ring parity 1 is zeros; cell1 state reset after w0)
                nc.tensor.matmul(
                    out=pz[:, :, s1, :], lhsT=w1s[:, wcol],
                    rhs=ring[:, rpar_prev, :, 0, :],
                    start=st(), stop=False, skip_group_check=True)
                nc.tensor.matmul(
                    out=pz[:, :, s1, :], lhsT=b1s[:, wcol],
                    rhs=ones[:], start=False, stop=False,
                    skip_group_check=True)
            return pz

        def step(w, s, pz, cells=(0, 1)):
            par = w % 2
            have0 = 0 in cells
            have1 = 1 in cells
            # g-chunks first so tanh_g can start while i,f,o matmuls run
            for j in (2, 0, 1, 3):
                s0, s1 = CHUNK_SLOT[j]
                wcol = slice(j * H, (j + 1) * H)
                if have0:
                    rhs0 = (ring[:, par, s - 1, 0, :] if s > 0
                            else ring[:, 1 - par, G - 1, 0, :])
                    nc.tensor.matmul(
                        out=pz[:, s, s0, :], lhsT=u0s[:, wcol], rhs=rhs0,
                        start=False, stop=(j == 3), skip_group_check=True)
                if have1:
                    rhs1 = (ring[:, par, s - 1, 1, :] if s > 0
                            else ring[:, 1 - par, G - 1, 1, :])
                    nc.tensor.matmul(
                        out=pz[:, s, s1, :], lhsT=u1s[:, wcol], rhs=rhs1,
                        start=False, stop=(j == 3), skip_group_check=True)

            cin = cA if s % 2 == 0 else cB
            cout = cB if s % 2 == 0 else cA
            if have0 and have1:
                csl = slice(0, 2)
                zsl_ifo = pz[:, s, 0:6, :]
                zsl_g = pz[:, s, 6:8, :]
                nsig = 6
            elif have0:
                csl = slice(0, 1)
                zsl_ifo = pz[:, s, 0:6:2, :]
                zsl_g = pz[:, s, 6:7, :]
                nsig = 3
            else:
                csl = slice(1, 2)
                zsl_ifo = pz[:, s, 1:6:2, :]
                zsl_g = pz[:, s, 7:8, :]
                nsig = 3
            ncl = nsig // 3

            tg = act.tile([H, 2, b], BF16, tag="tg")
            sig = act.tile([H, 6, b], BF16, tag="sig")
            tcv = act.tile([H, 2, b], BF16, tag="tc")
            t1 = tmp.tile([H, 2, b], F32, tag="t1")
            fc = tmp.tile([H, 2, b], F32, tag="fc")

            nc.scalar.activation(tg[:, :ncl, :], zsl_g, AF.Tanh)
            nc.scalar.activation(sig[:, :nsig, :], zsl_ifo, AF.Sigmoid)
            nc.vector.tensor_mul(t1[:, :ncl, :], sig[:, 0:ncl, :], tg[:, :ncl, :])
            eng = nc.gpsimd if use_gp else nc.vector
            eng.tensor_mul(fc[:, :ncl, :], sig[:, ncl:2 * ncl, :], cin[:, csl, :])
            nc.vector.tensor_add(cout[:, csl, :], fc[:, :ncl, :], t1[:, :ncl, :])
            nc.scalar.activation(tcv[:, :ncl, :], cout[:, csl, :], AF.Tanh)
            if have0 and have1:
                hout = ring[:, par, s, :, :]
            elif have0:
                hout = ring[:, par, s, 0:1, :]
            else:
                hout = ring[:, par, s, 1:2, :]
            nc.vector.tensor_mul(hout, sig[:, 2 * ncl:3 * ncl, :], tcv[:, :ncl, :])

        def copy_out(w):
            par = w % 2
            nc.vector.tensor_copy(
                ohb[:, (w - 1) * G:w * G, :], ring[:, par, :, 1, :])

        # head: wall window 0 (cell1 computes garbage, reset below)
        pz = fill_window(0)
        for s in range(G):
            step(0, s, pz, cells=(0, 1))
        nc.vector.memset(cA[:, 1, :], 0.0)
        nc.vector.memset(ring[:, 0, G - 1, 1, :], 0.0)

        for w in range(1, W):
            pzw = fill_window(w)
            for s in range(G):
                step(w, s, pzw, cells=(0, 1))
            copy_out(w)

        nc.sync.dma_start(out=fcs[0:1, :, :], in_=cA[:, 0, :])

        # tail: wall window W (cell1 only)
        pz = fill_window(W)
        for s in range(G):
            step(W, s, pz, cells=(1,))
        copy_out(W)
        nc.sync.dma_start(out=fcs[1:2, :, :], in_=cA[:, 1, :])

        nc.sync.dma_start(out=oh[:], in_=ohb[:])

    nc.compile()
    return nc


def pack_core_inputs(x_slice, lift_w, lift_b, w0, u0, b0, w1, u1, b1,
                     reverse):
    """x_slice [b, T, 88] fp32 -> in_map for one core."""
    bsz, T, _ = x_slice.shape
    if reverse:
        x_slice = x_slice[:, ::-1, :]
    xT = np.ascontiguousarray(x_slice.transpose(2, 1, 0))  # [88, T, b]
    xTa = np.concatenate([xT, np.ones((1, T, bsz), np.float32)], axis=0)
    m0 = np.concatenate([lift_w @ w0, (lift_b @ w0 + b0)[None, :]], axis=0)
    bf = ml_dtypes.bfloat16
    return {
        "xT": xTa.astype(bf), "m0": m0.astype(bf), "u0": u0.astype(bf),
        "w1": w1.astype(bf), "u1": u1.astype(bf),
        "b1v": b1.astype(bf)[None, :],
    }


def make_in_maps(inputs, b=B_CORE):
    x = np.asarray(inputs["inputs"], np.float32)
    B = x.shape[0]
    lift_w = np.asarray(inputs["lift_w"], np.float32)
    lift_b = np.asarray(inputs["lift_b"], np.float32)
    maps = []
    for d, rev in (("fw", False), ("bw", True)):
        args = tuple(np.asarray(inputs[k], np.float32) for k in
                     (f"w_{d}0", f"u_{d}0", f"b_{d}0",
                      f"w_{d}1", f"u_{d}1", f"b_{d}1"))
        for i in range(B // b):
            maps.append(pack_core_inputs(
                x[i * b:(i + 1) * b], lift_w, lift_b, *args, reverse=rev))
    return maps


def assemble(results, B=B_FULL, T=T_FULL, b=B_CORE):
    nfw = B // b
    outputs = np.empty((B, T, 2 * H), np.float32)
    final_state = np.empty((B, 2 * H), np.float32)
    for i in range(nfw):
        rows = slice(i * b, (i + 1) * b)
        ohf = np.asarray(results[i]["oh"], dtype=np.float32)   # [128,T,b]
        outputs[rows, :, :H] = ohf.transpose(2, 1, 0)
        fcsv = np.asarray(results[i]["fcs"])                    # [2,128,b]
        final_state[rows, :H] = fcsv[0].T
        final_state[rows, H:] = fcsv[1].T
        ohr = np.asarray(results[nfw + i]["oh"], dtype=np.float32)
        outputs[rows, :, H:] = ohr.transpose(2, 1, 0)[:, ::-1, :]
    return outputs, final_state


def kernel(**inputs):
    T = int(np.asarray(inputs["inputs"]).shape[1])
    key = (T,)
    if key not in _NC_CACHE:
        _NC_CACHE[key] = build(T=T)
    nc = _NC_CACHE[key]
    in_maps = make_in_maps(inputs)
    res = run_bass_kernel_spmd(nc, in_maps, core_ids=list(range(N_CORES)))
    return assemble(res.results, B=len(in_maps) // 2 * B_CORE, T=T)


# revision 7
# speedup vs baseline: 16.6672x; 16.6672x over previous
"""Bidirectional 2-layer LSTM encoder (B=64, T=2048, F=88, H=128) on 8
Trainium2 NeuronCores via Bass/Tile.

Sharding: data-parallel over batch x direction. Cores 0-3 run the forward
direction on 16 batch rows each; cores 4-7 run the backward direction
(time-reversed input) on the same row slices. Each core executes BOTH
layers of its direction as two staggered LSTM cells (layer 1 lags layer 0
by one G-step window), so there is no cross-core communication.

Per-core kernel (per step): 8 recurrent [128x128]@[128,b] bf16 matmuls
accumulate into a PSUM window tile [128, G, 8, b] pre-filled by batched
input GEMMs (lift fused into layer-0 weights on host; biases via a ones
row / rank-1 matmul); gates computed by 2 ACT ops (sigmoid over i,f,o of
both cells; tanh over g) + tanh(c); c/h updates on DVE/GPSIMD.
h is bf16; c accumulates in fp32.
"""

from contextlib import ExitStack

import numpy as np
import ml_dtypes

import concourse.bacc as bacc
import concourse.tile as tile
from concourse import mybir
from concourse.bass_utils import run_bass_kernel_spmd

BF16 = mybir.dt.bfloat16
F32 = mybir.dt.float32
AF = mybir.ActivationFunctionType

H = 128
FIN = 88
KX = FIN + 1  # x rows + ones row
B_FULL = 64
T_FULL = 2048
B_CORE = 16
N_CORES = 8

_NC_CACHE = {}


def build(T=T_FULL, b=B_CORE, G=4, use_gp=True, num_devices=N_CORES,
          T_buf=None, reps=1):
    """Build the per-core Bass program (fully unrolled). T_buf >= T keeps
    I/O shapes fixed while varying compute steps; reps > 1 re-runs the main
    window loop (timing probes only — output stays valid for reps=1)."""
    if T_buf is None:
        T_buf = T
    assert T % G == 0 and G % 2 == 0
    W = T // G  # cell0 windows; wall windows 0..W inclusive
    nc = bacc.Bacc("TRN2", target_bir_lowering=False, debug=False,
                   num_devices=num_devices)

    xT = nc.dram_tensor("xT", [KX, T_buf, b], BF16, kind="ExternalInput").ap()
    m0 = nc.dram_tensor("m0", [KX, 4 * H], BF16, kind="ExternalInput").ap()
    u0 = nc.dram_tensor("u0", [H, 4 * H], BF16, kind="ExternalInput").ap()
    w1 = nc.dram_tensor("w1", [H, 4 * H], BF16, kind="ExternalInput").ap()
    u1 = nc.dram_tensor("u1", [H, 4 * H], BF16, kind="ExternalInput").ap()
    b1v = nc.dram_tensor("b1v", [1, 4 * H], BF16, kind="ExternalInput").ap()
    oh = nc.dram_tensor("oh", [H, T_buf, b], BF16, kind="ExternalOutput").ap()
    fcs = nc.dram_tensor("fcs", [2, H, b], F32, kind="ExternalOutput").ap()

    # gate chunk (keras order i,f,g,o) -> (cell0 slot, cell1 slot)
    CHUNK_SLOT = {0: (0, 1), 1: (2, 3), 3: (4, 5), 2: (6, 7)}

    with tile.TileContext(nc) as tc, ExitStack() as ctx:
        wts = ctx.enter_context(tc.tile_pool(name="wts", bufs=1))
        state = ctx.enter_context(tc.tile_pool(name="state", bufs=1))
        big = ctx.enter_context(tc.tile_pool(name="big", bufs=1))
        psum = ctx.enter_context(tc.tile_pool(name="psum", bufs=3, space="PSUM"))
        act = ctx.enter_context(tc.tile_pool(name="act", bufs=4))
        tmp = ctx.enter_context(tc.tile_pool(name="tmp", bufs=4))

        m0s = wts.tile([KX, 4 * H], BF16)
        u0s = wts.tile([H, 4 * H], BF16)
        w1s = wts.tile([H, 4 * H], BF16)
        u1s = wts.tile([H, 4 * H], BF16)
        b1s = wts.tile([1, 4 * H], BF16)
        ones = wts.tile([1, G * b], BF16)
        nc.sync.dma_start(out=m0s[:], in_=m0[:])
        nc.sync.dma_start(out=u0s[:], in_=u0[:])
        nc.sync.dma_start(out=w1s[:], in_=w1[:])
        nc.sync.dma_start(out=u1s[:], in_=u1[:])
        nc.sync.dma_start(out=b1s[:], in_=b1v[:])
        nc.vector.memset(ones[:], 1.0)

        xs = big.tile([KX, T, b], BF16)
        nc.sync.dma_start(out=xs[:], in_=xT[:, :T, :])
        ohb = big.tile([H, T, b], BF16)

        ring = state.tile([H, 2, G, 2, b], BF16)  # [*, parity, step, cell, b]
        nc.vector.memset(ring[:], 0.0)
        cA = state.tile([H, 2, b], F32)
        cB = state.tile([H, 2, b], F32)
        nc.vector.memset(cA[:], 0.0)

        def fill_window(w):
            pz = psum.tile([H, G, 8, b], F32)
            t0 = w * G
            rpar_prev = (w - 1) % 2
            # start=True clears has_written for the WHOLE bank -> only the
            # first matmul into this tile may use it.
            first = [True]

            def st():
                v = first[0]
                first[0] = False
                return v

            for j in range(4):
                s0, s1 = CHUNK_SLOT[j]
                wcol = slice(j * H, (j + 1) * H)
                if w < W:
                    # x~0 for cell0 steps t0..t0+G (K=89 incl. bias row)
                    nc.tensor.matmul(
                        out=pz[:, :, s0, :], lhsT=m0s[:, wcol],
                        rhs=xs[:, t0:t0 + G, :],
                        start=st(), stop=False, skip_group_check=True)
                # x~1 for cell1 steps (w-1)G..wG from h0 history ring
                # (w=0: ring parity 1 is zeros; cell1 state reset after w0)
                nc.tensor.matmul(
                    out=pz[:, :, s1, :], lhsT=w1s[:, wcol],
                    rhs=ring[:, rpar_prev, :, 0, :],
                    start=st(), stop=False, skip_group_check=True)
                nc.tensor.matmul(
                    out=pz[:, :, s1, :], lhsT=b1s[:, wcol],
                    rhs=ones[:], start=False, stop=False,
                    skip_group_check=True)
            return pz

        def step(w, s, pz, cells=(0, 1)):
            par = w % 2
            have0 = 0 in cells
            have1 = 1 in cells
            # g-chunks first so tanh_g can start while i,f,o matmuls run
            for j in (2, 0, 1, 3):
                s0, s1 = CHUNK_SLOT[j]
                wcol = slice(j * H, (j + 1) * H)
                if have0:
                    rhs0 = (ring[:, par, s - 1, 0, :] if s > 0
                            else ring[:, 1 - par, G - 1, 0, :])
                    nc.tensor.matmul(
                        out=pz[:, s, s0, :], lhsT=u0s[:, wcol], rhs=rhs0,
                        start=False, stop=(j == 3), skip_group_check=True)
                if have1:
                    rhs1 = (ring[:, par, s - 1, 1, :] if s > 0
                            else ring[:, 1 - par, G - 1, 1, :])
                    nc.tensor.matmul(
                        out=pz[:, s, s1, :], lhsT=u1s[:, wcol], rhs=rhs1,
                        start=False, stop=(j == 3), skip_group_check=True)

            cin = cA if s % 2 == 0 else cB
            cout = cB if s % 2 == 0 else cA
            if have0 and have1:
                csl = slice(0, 2)
                zsl_ifo = pz[:, s, 0:6, :]
                zsl_g = pz[:, s, 6:8, :]
                nsig = 6
            elif have0:
                csl = slice(0, 1)
                zsl_ifo = pz[:, s, 0:6:2, :]
                zsl_g = pz[:, s, 6:7, :]
                nsig = 3
            else:
                csl = slice(1, 2)
                zsl_ifo = pz[:, s, 1:6:2, :]
                zsl_g = pz[:, s, 7:8, :]
                nsig = 3
            ncl = nsig // 3

            tg = act.tile([H, 2, b], BF16, tag="tg")
            sig = act.tile([H, 6, b], BF16, tag="sig")
            tcv = act.tile([H, 2, b], BF16, tag="tc")
            t1 = tmp.tile([H, 2, b], F32, tag="t1")
            fc = tmp.tile([H, 2, b], F32, tag="fc")

            nc.scalar.activation(tg[:, :ncl, :], zsl_g, AF.Tanh)
            nc.scalar.activation(sig[:, :nsig, :], zsl_ifo, AF.Sigmoid)
            nc.vector.tensor_mul(t1[:, :ncl, :], sig[:, 0:ncl, :], tg[:, :ncl, :])
            eng = nc.gpsimd if use_gp else nc.vector
            eng.tensor_mul(fc[:, :ncl, :], sig[:, ncl:2 * ncl, :], cin[:, csl, :])
            nc.vector.tensor_add(cout[:, csl, :], fc[:, :ncl, :], t1[:, :ncl, :])
            nc.scalar.activation(tcv[:, :ncl, :], cout[:, csl, :], AF.Tanh)
            if have0 and have1:
                hout = ring[:, par, s, :, :]
            elif have0:
                hout = ring[:, par, s, 0:1, :]
            else:
                hout = ring[:, par, s, 1:2, :]
            nc.vector.tensor_mul(hout, sig[:, 2 * ncl:3 * ncl, :], tcv[:, :ncl, :])

        def copy_out(w):
            par = w % 2
            nc.vector.tensor_copy(
                ohb[:, (w - 1) * G:w * G, :], ring[:, par, :, 1, :])

        # head: wall window 0 (cell1 computes garbage, reset below)
        pz = fill_window(0)
        for s in range(G):
            step(0, s, pz, cells=(0, 1))
        nc.vector.memset(cA[:, 1, :], 0.0)
        nc.vector.memset(ring[:, 0, G - 1, 1, :], 0.0)

        for _rep in range(reps):
            for w in range(1, W):
                pzw = fill_window(w)
                for s in range(G):
                    step(w, s, pzw, cells=(0, 1))
                copy_out(w)

        nc.sync.dma_start(out=fcs[0:1, :, :], in_=cA[:, 0, :])

        # tail: wall window W (cell1 only)
        pz = fill_window(W)
        for s in range(G):
            step(W, s, pz, cells=(1,))
        copy_out(W)
        nc.sync.dma_start(out=fcs[1:2, :, :], in_=cA[:, 1, :])

        nc.sync.dma_start(out=oh[:, :T, :], in_=ohb[:])

    nc.compile()
    return nc


def pack_core_inputs(x_slice, lift_w, lift_b, w0, u0, b0, w1, u1, b1,
                     reverse):
    """x_slice [b, T, 88] fp32 -> in_map for one core."""
    bsz, T, _ = x_slice.shape
    if reverse:
        x_slice = x_slice[:, ::-1, :]
    xT = np.ascontiguousarray(x_slice.transpose(2, 1, 0))  # [88, T, b]
    xTa = np.concatenate([xT, np.ones((1, T, bsz), np.float32)], axis=0)
    m0 = np.concatenate([lift_w @ w0, (lift_b @ w0 + b0)[None, :]], axis=0)
    bf = ml_dtypes.bfloat16
    return {
        "xT": xTa.astype(bf), "m0": m0.astype(bf), "u0": u0.astype(bf),
        "w1": w1.astype(bf), "u1": u1.astype(bf),
        "b1v": b1.astype(bf)[None, :],
    }


def make_in_maps(inputs, b=B_CORE):
    x = np.asarray(inputs["inputs"], np.float32)
    B = x.shape[0]
    lift_w = np.asarray(inputs["lift_w"], np.float32)
    lift_b = np.asarray(inputs["lift_b"], np.float32)
    maps = []
    for d, rev in (("fw", False), ("bw", True)):
        args = tuple(np.asarray(inputs[k], np.float32) for k in
                     (f"w_{d}0", f"u_{d}0", f"b_{d}0",
                      f"w_{d}1", f"u_{d}1", f"b_{d}1"))
        for i in range(B // b):
            maps.append(pack_core_inputs(
                x[i * b:(i + 1) * b], lift_w, lift_b, *args, reverse=rev))
    return maps


def assemble(results, B=B_FULL, T=T_FULL, b=B_CORE):
    nfw = B // b
    outputs = np.empty((B, T, 2 * H), np.float32)
    final_state = np.empty((B, 2 * H), np.float32)
    for i in range(nfw):
        rows = slice(i * b, (i + 1) * b)
        ohf = np.asarray(results[i]["oh"], dtype=np.float32)   # [128,T,b]
        outputs[rows, :, :H] = ohf.transpose(2, 1, 0)
        fcsv = np.asarray(results[i]["fcs"])                    # [2,128,b]
        final_state[rows, :H] = fcsv[0].T
        final_state[rows, H:] = fcsv[1].T
        ohr = np.asarray(results[nfw + i]["oh"], dtype=np.float32)
        outputs[rows, :, H:] = ohr.transpose(2, 1, 0)[:, ::-1, :]
    return outputs, final_state


def kernel(**inputs):
    T = int(np.asarray(inputs["inputs"]).shape[1])
    key = (T,)
    if key not in _NC_CACHE:
        _NC_CACHE[key] = build(T=T)
    nc = _NC_CACHE[key]
    in_maps = make_in_maps(inputs)
    res = run_bass_kernel_spmd(nc, in_maps, core_ids=list(range(N_CORES)))
    return assemble(res.results, B=len(in_maps) // 2 * B_CORE, T=T)
